# revision 20
# baseline (speedup 1.0000x reference)
# HEPOS cross-attention (strided per-head K/V) on 8 Trainium2 NeuronCores.
#
# Reference computation (per head h, stride s = STRIDE_LIST[h]):
#   Q = x @ Wq.T + bq ; K = e @ Wk.T + bk ; V = e @ Wv.T + bv
#   out_h = softmax(Q_h @ K_h[::s].T / 8) @ V_h[::s]
#   out   = concat_h(out_h) @ Wo.T + bo
#
# Sharding: 64 (batch, head) units over 8 cores. Core c owns head group
# g = c % 4 (heads 4g..4g+3, strides [1,2,4,8]) and batch pair [0,1]
# (c < 4) or [2,3] (c >= 4). Each core computes its heads' contribution
# to out; the host sums the four partials per batch and adds bo.
#
# On-device design (v3):
#  * Heads are processed as two stride PAIRS (sA, 2*sA): (1,2) and (4,8).
#    Head A of a pair lives on SBUF partitions 0-63, head B on 64-127.
#  * Q/K/V projections use the weight matrix as the matmul stationary with
#    both heads packed into the 128 stationary columns (full PE width).
#    K^T/V^T stream the "union" encoder columns (stride sA); head B rows
#    are valid at even union columns and are compacted on evacuation.
#    The stride-4 union for pair (4,8) is pre-packed by the host (eT4).
#  * V^T ([hd, keys]) is flipped to AV orientation ([keys, hd]) with ONE
#    DMA XBAR transpose per (block, head) (3D output access pattern) -
#    zero PE cost, one sync-engine dispatch each.
#  * Scores are computed transposed ([keys, T]); head B's score matmuls
#    use partitions 64-127 (PE row-tile T8) and overlap head A's (T0).
#  * AV accumulates into PSUM tiles resident across all encoder blocks of
#    a (batch, pair); the softmax denominator falls out of a ones-column
#    appended to the V stationary.
#  * Scores of chunk k+1 are issued before AV of chunk k so the PE never
#    waits on the scalar engine's exp.
#  * All DRAM->SBUF loads are single merged DMAs ([128, ndc, *] access
#    patterns); PSUM score tiles are bank-pair wide ([128, 2*tt]) so exp
#    and evacuations run as one instruction per tile.

import os
import sys

import ml_dtypes
import numpy as np

BF16 = ml_dtypes.bfloat16

for _p in ("/opt/trn_rl_repo", "/root/.axon_site/_ro/trn_rl_repo"):
    if os.path.isdir(_p) and _p not in sys.path:
        sys.path.insert(0, _p)

import concourse.bass as bass  # noqa: E402
import concourse.tile as tile  # noqa: E402
from concourse import bacc, mybir  # noqa: E402
from concourse import bass_utils  # noqa: E402

F32 = mybir.dt.float32
MM_DT = mybir.dt.bfloat16  # matmul operand dtype: full PE rate, half DMA
AF = mybir.ActivationFunctionType

D_MODEL = 1024
NUM_HEADS = 16
HEAD_DIM = 64
STRIDE_LIST = [1, 2, 4, 8] * 4
B, T, S = 4, 1024, 4096
N_CORES = 8

FULL_CFG = dict(
    nb=2,  # batches per core
    T=T,
    S=S,
    D=D_MODEL,
    strides=(1, 2, 4, 8),  # per-core head strides; pairs (s0,s1),(s2,s3)
    hd=HEAD_DIM,
    blk=1024,  # encoder S-block (stride-1 columns) per iteration
    tt=512,  # T tile (PSUM free-dim limit for fp32)
)

FR = MM_DT
WHOLE_BLOCK_TRANSPOSE = True


def _mm(nc, out, lhsT, rhs, start, stop):
    nc.tensor.matmul(out, lhsT, rhs, start=start, stop=stop)


def build_program(cfg):
    """Build the per-core Bass/Tile program (same program on all cores)."""
    nb, Tl, Sl, Dl = cfg["nb"], cfg["T"], cfg["S"], cfg["D"]
    strides, hd = cfg["strides"], cfg["hd"]
    assert strides[1] == 2 * strides[0] and strides[3] == 2 * strides[2]
    HP = 4 * hd  # packed head rows (256)
    s4 = strides[2]

    nc = bacc.Bacc(
        "TRN2",
        target_bir_lowering=False,
        debug=False,
        enable_asserts=False,
        num_devices=N_CORES,
    )

    xT = nc.dram_tensor("xT", [Dl, nb * Tl], MM_DT, kind="ExternalInput").ap()
    eT = nc.dram_tensor("eT", [Dl, nb * Sl], MM_DT, kind="ExternalInput").ap()
    eT4 = nc.dram_tensor(
        "eT4", [Dl, nb * (Sl // s4)], MM_DT, kind="ExternalInput"
    ).ap()
    wqT = nc.dram_tensor("wqT", [Dl, HP], MM_DT, kind="ExternalInput").ap()
    wkT = nc.dram_tensor("wkT", [Dl, HP], MM_DT, kind="ExternalInput").ap()
    wvT = nc.dram_tensor("wvT", [Dl, HP], MM_DT, kind="ExternalInput").ap()
    woT = nc.dram_tensor("woT", [HP, Dl], MM_DT, kind="ExternalInput").ap()
    biases = nc.dram_tensor("biases", [6, 128, 1], F32, kind="ExternalInput").ap()
    out = nc.dram_tensor("partial", [nb * Tl, Dl], F32, kind="ExternalOutput").ap()

    with tile.TileContext(nc) as tc:
        _build_tile(tc, cfg, xT, eT, eT4, wqT, wkT, wvT, woT, biases, out)

    nc.compile()
    return nc


def _build_tile(tc, cfg, xT, eT, eT4, wqT, wkT, wvT, woT, biases, out):
    nc = tc.nc
    nb, Tl, Sl, Dl = cfg["nb"], cfg["T"], cfg["S"], cfg["D"]
    strides, hd = cfg["strides"], cfg["hd"]
    blk, tt = cfg["blk"], cfg["tt"]
    ndc = Dl // 128
    nblk = Sl // blk
    ntt = Tl // tt
    assert ntt == 2, "wide PSUM tiles assume T == 2*tt"
    HP = 4 * hd
    scale = 1.0 / float(np.sqrt(hd))

    from contextlib import ExitStack

    with ExitStack() as ctx:
        wpool = ctx.enter_context(tc.tile_pool(name="weights", bufs=1))
        qtpool = ctx.enter_context(tc.tile_pool(name="qt", bufs=1))
        etpool = ctx.enter_context(tc.tile_pool(name="et", bufs=3))
        ktpool = ctx.enter_context(tc.tile_pool(name="kt", bufs=3))
        vtpool = ctx.enter_context(tc.tile_pool(name="vtT", bufs=2))
        vpool = ctx.enter_context(tc.tile_pool(name="v", bufs=3))
        ppool = ctx.enter_context(tc.tile_pool(name="p", bufs=4))
        npool = ctx.enter_context(tc.tile_pool(name="norm", bufs=2))
        otpool = ctx.enter_context(tc.tile_pool(name="ot", bufs=2))
        obpool = ctx.enter_context(tc.tile_pool(name="outs", bufs=3))
        # PSUM: sc/kv are bank-pair wide ([128, 2*tt] fp32 = 2 banks each),
        # av holds 4 single-bank accumulators -> 8 banks total.
        sc_ps = ctx.enter_context(tc.tile_pool(name="sc_ps", bufs=1, space="PSUM"))
        kv_ps = ctx.enter_context(tc.tile_pool(name="kv_ps", bufs=1, space="PSUM"))
        av_ps = ctx.enter_context(tc.tile_pool(name="av_ps", bufs=1, space="PSUM"))

        # ---- weights into SBUF (one DMA per tensor) ----
        wq_sb = wpool.tile([128, ndc * HP], FR, tag="wq", name="wq_sb")
        wk_sb = wpool.tile([128, ndc * HP], FR, tag="wk", name="wk_sb")
        wv_sb = wpool.tile([128, ndc * HP], FR, tag="wv", name="wv_sb")
        wo_sb = wpool.tile([128, 2 * Dl], FR, tag="wo", name="wo_sb")
        bias_sb = wpool.tile([128, 6], F32, tag="bias", name="bias_sb")
        ones_sb = wpool.tile([128, 1], F32, tag="ones", name="ones_sb")
        den_sb = wpool.tile([97, tt], F32, tag="den", name="den_sb")

        def wslice(wsb, dc, p):
            return wsb[:, dc * HP + p * 128 : dc * HP + (p + 1) * 128]

        # encoder block list + DMA helper (defined early so the first
        # block's load can be interleaved with the weight loads).
        # Blocks are uniform in UNION columns (blk per block) so pair (4,8)
        # gets one full-sized block instead of four tiny ones.
        def block_params(p):
            sA = strides[2 * p]
            Scols = Sl // sA
            return dict(
                src=eT if p == 0 else eT4,
                Scols=Scols,
                nblk_p=max(1, Scols // blk),
            )

        blocks = [
            (b, p, ib)
            for b in range(nb)
            for p in range(2)
            for ib in range(block_params(p)["nblk_p"])
        ]

        def block_ublk(p, ib):
            bp = block_params(p)
            return min(blk, bp["Scols"] - ib * blk)

        def emit_et_dma(b, p, ib):
            bp = block_params(p)
            ublk = block_ublk(p, ib)
            et = etpool.tile([128, ndc * ublk], FR, tag="et", name="et_t")
            c0_ = b * bp["Scols"] + ib * blk
            nc.sync.dma_start(
                out=et.rearrange("p (c u) -> p c u", c=ndc),
                in_=bp["src"][:, c0_ : c0_ + ublk].rearrange(
                    "(c p) u -> p c u", p=128
                ),
            )
            return et

        nc.sync.dma_start(
            out=wq_sb.rearrange("p (c h) -> p c h", c=ndc),
            in_=wqT.rearrange("(c p) h -> p c h", p=128),
        )
        xts = {}
        with tc.tile_pool(name="xt", bufs=1) as xpool:
            for b in range(nb):
                xt = xpool.tile([128, ndc * Tl], FR, tag=f"xt{b}", name="xt")
                xts[b] = xt
            nc.sync.dma_start(
                out=xts[0].rearrange("p (c t) -> p c t", c=ndc),
                in_=xT[:, 0:Tl].rearrange("(c p) t -> p c t", p=128),
            )
            nc.sync.dma_start(
                out=wk_sb.rearrange("p (c h) -> p c h", c=ndc),
                in_=wkT.rearrange("(c p) h -> p c h", p=128),
            )
            et_next = emit_et_dma(*blocks[0])
            nc.sync.dma_start(
                out=wv_sb.rearrange("p (c h) -> p c h", c=ndc),
                in_=wvT.rearrange("(c p) h -> p c h", p=128),
            )
            for b in range(1, nb):
                nc.sync.dma_start(
                    out=xts[b].rearrange("p (c t) -> p c t", c=ndc),
                    in_=xT[:, b * Tl : (b + 1) * Tl].rearrange(
                        "(c p) t -> p c t", p=128
                    ),
                )
            nc.sync.dma_start(
                out=wo_sb.rearrange("p (g d) -> p g d", g=2),
                in_=woT.rearrange("(g p) d -> p g d", p=128),
            )
            nc.sync.dma_start(
                out=bias_sb, in_=biases.rearrange("g p one -> p (g one)")
            )
            nc.vector.memset(ones_sb, 1.0)

            # PE warm-up: ~25 dependency-free matmuls on a zeroed tile keep
            # the PE busy while the first DMAs land, so the HAM clock gate
            # opens (1.2 -> 2.4 GHz) before the real matmul stream begins.
            warm = wpool.tile([128, tt], FR, tag="warm", name="warm")
            nc.vector.memset(warm, 0.0)
            nc.vector.memset(den_sb, 1.0)
            wps = kv_ps.tile([128, 2 * tt], F32, tag="kv", name="kv_psum")
            for _ in range(50):
                _mm(nc, wps[:, 0:tt], warm[:, 0:128], warm, start=True, stop=True)

            # ---- phase 1: Q^T = (x @ Wq.T + bq)^T, head pairs on partitions
            qt_sb = {}  # (b, pair) -> [128, T] tile
            for b in range(nb):
                for p in range(2):
                    qt = qtpool.tile([128, Tl], FR, tag=f"qt{b}{p}", name="qt")
                    qt_sb[(b, p)] = qt
                    ps = sc_ps.tile([128, 2 * tt], F32, tag="sc", name="sc_psum")
                    for nt in range(ntt):
                        for dc in range(ndc):
                            _mm(
                                nc,
                                ps[:, nt * tt : (nt + 1) * tt],
                                wslice(wq_sb, dc, p),
                                xts[b][:, dc * Tl + nt * tt : dc * Tl + (nt + 1) * tt],
                                start=(dc == 0),
                                stop=(dc == ndc - 1),
                            )
                    nc.scalar.activation(
                        qt, ps, AF.Identity, bias=bias_sb[:, p : p + 1]
                    )

        # ---- phase 2: attention per (batch, pair), out proj per batch ----
        # The per-block work is split into phase A (K^T/V^T projection,
        # evacuation, V transposes, next-block encoder DMA) and phase B
        # (scores/exp/AV chunk loop), software-pipelined one block deep:
        #   pA(0) pA(1) pB(0) pA(2) pB(1) ... pA(n-1) pB(n-3) pB(n-2) pB(n-1)
        # so V transposes are dispatched a full block before their AV
        # consumes them and the normalize chain never blocks evacuations.
        # AV emission inside phase B additionally lags scores by two chunk
        # steps so the PE never waits on the scalar engine's exp.
        assert Dl <= 2 * tt
        ot_sb = {}
        avp_live = {}
        blk_state = {}
        pending = []  # (age, avp, vt, pt, h, first, last)

        def flush_pending(min_age=2):
            keep = []
            for age, avp, vt, pt, h, first, last in pending:
                if age >= min_age:
                    for nt in range(ntt):
                        _mm(
                            nc,
                            avp[(h, nt)],
                            vt,
                            pt[:, nt * tt : (nt + 1) * tt],
                            start=first,
                            stop=last,
                        )
                else:
                    keep.append((age + 1, avp, vt, pt, h, first, last))
            pending[:] = keep

        def emit_avp_copies(b, p):
            """Copy the AV accumulators PSUM -> SBUF with fast scalar-engine
            copies so the PSUM banks free up for the next pair immediately."""
            avp = avp_live.pop((b, p))
            avc = {}
            for nt in range(ntt):
                for h in range(2):
                    c = npool.tile([hd + 1, tt], F32, tag=f"avc{h}{nt}", name="avc")
                    nc.scalar.copy(c, avp[(h, nt)])
                    avc[(h, nt)] = c
            return avc

        def emit_norm_rest(b, p, avc):
            """Reciprocal chain out of SBUF; emitted AFTER the next phase_a
            so it never delays pipeline-critical evacuations. The four
            denominators are stacked on partitions {0,32,64,96} (the legal
            start partitions) so ONE reciprocal instruction serves all four
            (h, nt) chains - reciprocal cost is per-lane-serial in the free
            dim, so batching across partitions is free."""
            ot = otpool.tile([128, Tl], FR, tag=f"ot{p}", name="ot")
            ot_sb[(b, p)] = ot
            order = [(h, nt) for nt in range(ntt) for h in range(2)]
            for i, (h, nt) in enumerate(order):
                nc.vector.tensor_copy(
                    den_sb[32 * i : 32 * i + 1, :], avc[(h, nt)][hd : hd + 1, :]
                )
            rcp = npool.tile([97, tt], F32, tag="rcp", name="rcp")
            nc.vector.reciprocal(rcp, den_sb)
            for i, (h, nt) in enumerate(order):
                r0 = npool.tile([1, tt], F32, tag="r0", name="r0")
                nc.vector.tensor_copy(r0, rcp[32 * i : 32 * i + 1, :])
                rb = npool.tile([hd, tt], F32, tag="rb", name="rbcast")
                nc.gpsimd.partition_broadcast(rb, r0)
                nc.vector.tensor_mul(
                    ot[h * hd : (h + 1) * hd, nt * tt : (nt + 1) * tt],
                    avc[(h, nt)][0:hd, :],
                    rb,
                )

        def emit_out_proj(b):
            for tc_i in range(Tl // 128):
                pool, tg = (sc_ps, "sc") if tc_i % 2 == 0 else (kv_ps, "kv")
                ops = pool.tile([128, 2 * tt], F32, tag=tg, name="o_psum")
                for j in range(0, Dl, tt):
                    dw = min(tt, Dl - j)
                    for p in range(2):
                        _mm(
                            nc,
                            ops[:, j : j + dw],
                            ot_sb[(b, p)][:, tc_i * 128 : (tc_i + 1) * 128],
                            wo_sb[:, p * Dl + j : p * Dl + j + dw],
                            start=(p == 0),
                            stop=(p == 1),
                        )
                ob = obpool.tile([128, Dl], F32, tag="ob", name="ob")
                nc.vector.tensor_copy(ob, ops[:, 0:Dl])
                nc.sync.dma_start(
                    out=out[b * Tl + tc_i * 128 : b * Tl + (tc_i + 1) * 128, :],
                    in_=ob,
                )

        ets = {0: et_next}

        def phase_a(bi):
            b, p, ib = blocks[bi]
            ublk = block_ublk(p, ib)
            nA = ublk // 128
            nB = nA // 2
            et = ets.pop(bi)
            # K^T proj (packed pair; B compacted to even union cols)
            kt_A = ktpool.tile([64, ublk], FR, tag="ktA", name="kt_A")
            kt_B = ktpool.tile([128, ublk // 2], FR, tag="ktB", name="kt_B")
            vtT_A = vtpool.tile([64, ublk], FR, tag="vtA", name="vtT_A")
            vtT_B = vtpool.tile([128, ublk // 2], FR, tag="vtB", name="vtT_B")
            kps = sc_ps.tile([128, 2 * tt], F32, tag="sc", name="sc_psum")
            for c0 in range(0, ublk, tt):
                cw = min(tt, ublk - c0)
                for dc in range(ndc):
                    _mm(
                        nc,
                        kps[:, c0 : c0 + cw],
                        wslice(wk_sb, dc, p),
                        et[:, dc * ublk + c0 : dc * ublk + c0 + cw],
                        start=(dc == 0),
                        stop=(dc == ndc - 1),
                    )
            nc.vector.tensor_scalar_add(
                kt_A, kps[0:64, 0:ublk], bias_sb[0:64, 2 + p : 3 + p]
            )
            nc.vector.tensor_scalar_add(
                kt_B[64:128, :],
                kps[64:128, 0:ublk:2],
                bias_sb[64:128, 2 + p : 3 + p],
            )
            # V^T proj (same streaming, wv stationary)
            vps = kv_ps.tile([128, 2 * tt], F32, tag="kv", name="kv_psum")
            for c0 in range(0, ublk, tt):
                cw = min(tt, ublk - c0)
                for dc in range(ndc):
                    _mm(
                        nc,
                        vps[:, c0 : c0 + cw],
                        wslice(wv_sb, dc, p),
                        et[:, dc * ublk + c0 : dc * ublk + c0 + cw],
                        start=(dc == 0),
                        stop=(dc == ndc - 1),
                    )
            nc.vector.tensor_scalar_add(
                vtT_A, vps[0:64, 0:ublk], bias_sb[0:64, 4 + p : 5 + p]
            )
            nc.vector.tensor_scalar_add(
                vtT_B[64:128, :],
                vps[64:128, 0:ublk:2],
                bias_sb[64:128, 4 + p : 5 + p],
            )

            # prefetch the NEXT block's encoder tile before the transposes
            if bi + 1 < len(blocks):
                ets[bi + 1] = emit_et_dma(*blocks[bi + 1])

            # V -> [keys, hd] via per-chunk DMA XBAR transposes, dispatched
            # in consumption order (A0 A1 B0 A2 A3 B1 ...). Chunk pitch 80
            # elems (160B) keeps destinations 32B-aligned (xbar encoding).
            VP = hd + 16
            vtA = vpool.tile([128, nA * VP], FR, tag="vA", name="vtA")
            vtA3 = vtA.rearrange("p (c f) -> p c f", c=nA)
            nc.vector.memset(vtA3[:, :, hd : hd + 1], 1.0)
            vtB = vpool.tile([128, nB * VP], FR, tag="vB", name="vtB")
            vtB3 = vtB.rearrange("p (c f) -> p c f", c=nB)
            nc.vector.memset(vtB3[:, :, hd : hd + 1], 1.0)
            if WHOLE_BLOCK_TRANSPOSE:
                nc.sync.dma_start(out=vtA3[:, :, 0:hd], in_=vtT_A, transpose=True)
                nc.sync.dma_start(
                    out=vtB3[:, :, 0:hd], in_=vtT_B[64:128, :], transpose=True
                )
            else:
                for ck in range(nA):
                    nc.sync.dma_start(
                        out=vtA3[:, ck, 0:hd],
                        in_=vtT_A[:, ck * 128 : (ck + 1) * 128],
                        transpose=True,
                    )
                    if ck % 2 == 1:
                        ckb = ck // 2
                        nc.sync.dma_start(
                            out=vtB3[:, ckb, 0:hd],
                            in_=vtT_B[64:128, ckb * 128 : (ckb + 1) * 128],
                            transpose=True,
                        )
            blk_state[bi] = (kt_A, kt_B, vtA, vtB, nA, nB)

        def phase_b(bi):
            b, p, ib = blocks[bi]
            kt_A, kt_B, vtA, vtB, nA, nB = blk_state.pop(bi)
            nblk_p = block_params(p)["nblk_p"]
            VP = hd + 16
            if (b, p) not in avp_live:
                avp_live[(b, p)] = {
                    (h, nt): av_ps.tile(
                        [hd + 1, tt], F32, tag=f"av{h}{nt}", name="av_psum"
                    )
                    for h in range(2)
                    for nt in range(ntt)
                }
            avp = avp_live[(b, p)]

            for ck in range(nA):
                do_B = ck % 2 == 1
                ckb = ck // 2
                ptA = ppool.tile([128, Tl], FR, tag="pA", name="ptA")
                sa = sc_ps.tile([128, 2 * tt], F32, tag="sc", name="sc_psum")
                if do_B:
                    ptB = ppool.tile([128, Tl], FR, tag="pB", name="ptB")
                    sb_ = kv_ps.tile([128, 2 * tt], F32, tag="kv", name="kv_psum")
                for nt in range(ntt):
                    _mm(
                        nc,
                        sa[:, nt * tt : (nt + 1) * tt],
                        kt_A[:, ck * 128 : (ck + 1) * 128],
                        qt_sb[(b, p)][0:64, nt * tt : (nt + 1) * tt],
                        start=True,
                        stop=True,
                    )
                    if do_B:
                        _mm(
                            nc,
                            sb_[:, nt * tt : (nt + 1) * tt],
                            kt_B[64:128, ckb * 128 : (ckb + 1) * 128],
                            qt_sb[(b, p)][64:128, nt * tt : (nt + 1) * tt],
                            start=True,
                            stop=True,
                        )
                nc.scalar.activation(ptA, sa, AF.Exp, scale=scale)
                if do_B:
                    nc.scalar.activation(ptB, sb_, AF.Exp, scale=scale)
                flush_pending()
                pending.append(
                    (
                        0,
                        avp,
                        vtA[:, ck * VP : ck * VP + hd + 1],
                        ptA,
                        0,
                        ib == 0 and ck == 0,
                        ib == nblk_p - 1 and ck == nA - 1,
                    )
                )
                if do_B:
                    pending.append(
                        (
                            0,
                            avp,
                            vtB[:, ckb * VP : ckb * VP + hd + 1],
                            ptB,
                            1,
                            ib == 0 and ckb == 0,
                            ib == nblk_p - 1 and ckb == nB - 1,
                        )
                    )

            if ib == nblk_p - 1:
                flush_pending(min_age=0)
                norm_todo.append((b, p, emit_avp_copies(b, p)))
            if b > 0 and p == 0 and ib == 0:
                emit_out_proj(b - 1)  # deferred past the previous normalize

        nbl = len(blocks)
        norm_todo = []
        phase_a(0)
        if nbl > 1:
            phase_a(1)
        for i in range(nbl):
            phase_b(i)
            if i + 2 < nbl:
                phase_a(i + 2)
            while norm_todo:
                nb_, np_, avc_ = norm_todo.pop(0)
                emit_norm_rest(nb_, np_, avc_)
        emit_out_proj(nb - 1)

# ---------------------------------------------------------------------------
# Host-side sharding / gathering
# ---------------------------------------------------------------------------


def _core_map():
    """core -> (batches, heads)"""
    m = {}
    for c in range(N_CORES):
        g = c % 4
        bs = [0, 1] if c < 4 else [2, 3]
        hs = [4 * g + i for i in range(4)]
        m[c] = (bs, hs)
    return m


def shard_inputs(inputs, cfg):
    x = np.asarray(inputs["decoder_input"], np.float32)
    e = np.asarray(inputs["encoder_output"], np.float32)
    Wq = np.asarray(inputs["Wq"], np.float32)
    Wk = np.asarray(inputs["Wk"], np.float32)
    Wv = np.asarray(inputs["Wv"], np.float32)
    Wo = np.asarray(inputs["Wo"], np.float32)
    bq = np.asarray(inputs["bq"], np.float32)
    bk = np.asarray(inputs["bk"], np.float32)
    bv = np.asarray(inputs["bv"], np.float32)
    hd = cfg["hd"]
    s4 = cfg["strides"][2]
    in_maps = []
    for c, (bs, hs) in _core_map().items():
        rows = np.concatenate([np.arange(h * hd, (h + 1) * hd) for h in hs])
        xTc = np.ascontiguousarray(
            x[bs].reshape(len(bs) * cfg["T"], cfg["D"]).T.astype(BF16)
        )
        eTc = np.ascontiguousarray(
            e[bs].reshape(len(bs) * cfg["S"], cfg["D"]).T.astype(BF16)
        )
        e4 = e[bs][:, ::s4, :]  # [nb, S//s4, D]
        eT4c = np.ascontiguousarray(
            e4.reshape(len(bs) * (cfg["S"] // s4), cfg["D"]).T.astype(BF16)
        )
        bias = np.stack([bq[rows], bk[rows], bv[rows]]).reshape(6, 128, 1)
        in_maps.append(
            {
                "xT": xTc,
                "eT": eTc,
                "eT4": eT4c,
                "wqT": np.ascontiguousarray(Wq[rows].T.astype(BF16)),
                "wkT": np.ascontiguousarray(Wk[rows].T.astype(BF16)),
                "wvT": np.ascontiguousarray(Wv[rows].T.astype(BF16)),
                "woT": np.ascontiguousarray(Wo[:, rows].T.astype(BF16)),
                "biases": np.ascontiguousarray(bias),
            }
        )
    return in_maps


def gather_output(results, bo, cfg):
    Tl, Dl = cfg["T"], cfg["D"]
    out = np.zeros((B, Tl, Dl), np.float32)
    for c, (bs, _hs) in _core_map().items():
        p = results[c]["partial"].reshape(len(bs), Tl, Dl)
        for i, b in enumerate(bs):
            out[b] += p[i]
    return out + np.asarray(bo, np.float32)[None, None, :]


_COMPILED = None


def _get_compiled():
    global _COMPILED
    if _COMPILED is None:
        _COMPILED = build_program(FULL_CFG)
    return _COMPILED


def run_on_cores(inputs, trace=False, **kw):
    nc = _get_compiled()
    in_maps = shard_inputs(inputs, FULL_CFG)
    res = bass_utils.run_bass_kernel_spmd(
        nc, in_maps, core_ids=list(range(N_CORES)), trace=trace, **kw
    )
    return res


def kernel(**inputs) -> np.ndarray:
    res = run_on_cores(inputs, trace=False)
    return gather_output(res.results, inputs["bo"], FULL_CFG)
